# revision 22
# baseline (speedup 1.0000x reference)
"""ArcFace loss on 8 Trainium2 NeuronCores.

Sharding: batch data-parallel. Each core gets 1024 rows of the (normalized,
pre-scaled, transposed) input, the full normalized transposed weight, and
computes its [1024, 16384] slab of logits plus per-row sum(exp(logit-30)).
Host assembles the slabs, applies the ArcFace margin at the 8192 label
positions, and finishes the (tiny) log-softmax NLL reduction.
"""

import os
import sys

import numpy as np

for _p in ("/opt/trn_rl_repo",):
    if os.path.isdir(_p) and _p not in sys.path:
        sys.path.insert(0, _p)

def _ensure_ntff_hook():
    """Register antenv.axon_hooks (absent in the trimmed repo) so that
    run_bass_kernel_spmd(trace=True) can NTFF-profile under axon."""
    try:
        from antenv.axon_hooks import get_axon_ntff_profile_hook  # noqa: F401

        return
    except ImportError:
        pass
    import types

    import antenv

    mod = types.ModuleType("antenv.axon_hooks")
    mod._hook = None

    def set_axon_ntff_profile_hook(h):
        mod._hook = h

    def get_axon_ntff_profile_hook():
        if mod._hook is None:
            try:
                from trn_agent_boot.trn_boot import _ntff_profile_via_ctypes

                mod._hook = _ntff_profile_via_ctypes("/opt/axon/libaxon_pjrt.so")
            except Exception:
                return None
        return mod._hook

    mod.set_axon_ntff_profile_hook = set_axon_ntff_profile_hook
    mod.get_axon_ntff_profile_hook = get_axon_ntff_profile_hook
    sys.modules["antenv.axon_hooks"] = mod
    antenv.axon_hooks = mod


_ensure_ntff_hook()

import concourse.bass as bass  # noqa: E402
import concourse.mybir as mybir  # noqa: E402
import concourse.tile as tile  # noqa: E402
from concourse import bacc  # noqa: E402
from concourse.bass_utils import run_bass_kernel_spmd  # noqa: E402

import ml_dtypes  # noqa: E402

B, C, D = 8192, 16384, 256
NCORES = 8
BL = B // NCORES  # 1024 rows per core
P = 128
KT = D // P  # 2 contraction tiles of 128
NT = 512  # matmul moving-operand max (PSUM f32: one bank per matmul)
GROUP = 4  # class tiles per PSUM group
GW = NT * GROUP  # 2048 = one PSUM tile / one DMA-out chunk
SCALE = 30.0
MARGIN = 0.3
EPS = 1e-12

F32 = mybir.dt.float32
BF16 = mybir.dt.bfloat16
NP_BF16 = ml_dtypes.bfloat16

LAST_RESULTS = None  # BassKernelResults of the most recent run (for profiling)

_NC_CACHE = {}


def _build(bl, c):
    """Build + compile the per-core Bass graph for a [bl, c] logits slab."""
    mt = bl // P
    ng = c // GW
    nc = bacc.Bacc(
        "TRN2", target_bir_lowering=False, debug=False, num_devices=NCORES
    )
    xT = nc.dram_tensor("xT", [KT, P, bl], BF16, kind="ExternalInput")
    wT = nc.dram_tensor("wT", [KT, P, c], BF16, kind="ExternalInput")
    logits = nc.dram_tensor("logits", [bl, c], BF16, kind="ExternalOutput")
    sums = nc.dram_tensor("sums", [P, mt], F32, kind="ExternalOutput")

    with tile.TileContext(nc) as tc:
        with (
            tc.tile_pool(name="w", bufs=1) as wpool,
            tc.tile_pool(name="x", bufs=1) as xpool,
            tc.tile_pool(name="lg", bufs=10) as lpool,
            tc.tile_pool(name="ex", bufs=3) as epool,
            tc.tile_pool(name="acc", bufs=1) as apool,
            tc.tile_pool(name="ps", bufs=2, space="PSUM") as pspool,
        ):
            xk = [xpool.tile([P, bl], BF16, tag=f"x{k}", name=f"x{k}") for k in range(KT)]
            for k in range(KT):
                nc.sync.dma_start(xk[k][:], xT[k])
            # Resident normalized-transposed weights: KT x ng chunks of [P, GW]
            wk = [
                [wpool.tile([P, GW], BF16, tag=f"w{k}_{g}", name=f"w{k}_{g}") for g in range(ng)]
                for k in range(KT)
            ]
            for g in range(ng):
                for k in range(KT):
                    if g == 0:
                        # finer chunks so the first matmuls start sooner
                        for j in range(GROUP):
                            nc.sync.dma_start(
                                wk[k][g][:, j * NT : (j + 1) * NT],
                                wT[k, :, g * GW + j * NT : g * GW + (j + 1) * NT],
                            )
                    else:
                        nc.sync.dma_start(
                            wk[k][g][:], wT[k, :, g * GW : (g + 1) * GW]
                        )

            neg_scale = apool.tile([P, 1], F32, tag="neg_scale")
            nc.vector.memset(neg_scale[:], -SCALE)

            sums_sb = apool.tile([P, mt], F32, tag="sums")
            # SLAB groups of GW columns form one exp/DMA slab of SW columns
            SLAB = 1
            SW = GW * SLAB  # 2048
            ns = c // SW  # slabs per row-block
            accums = [
                apool.tile([P, ns], F32, tag=f"accum{m}", name=f"accum{m}")
                for m in range(mt)
            ]

            # Offload a few PSUM->SBUF copies to ACT to balance DVE/ACT load
            copy_idx = 0
            OFFLOAD_EVERY = 43

            for m in range(mt):
                for s in range(ns):
                    lg = lpool.tile([P, SW], BF16, tag="lg")
                    for q in range(SLAB):
                        g = s * SLAB + q
                        ps = pspool.tile([P, GW], F32, tag="ps")
                        for j in range(GROUP):
                            for k in range(KT):
                                nc.tensor.matmul(
                                    ps[:, j * NT : (j + 1) * NT],
                                    lhsT=xk[k][:, m * P : (m + 1) * P],
                                    rhs=wk[k][g][:, j * NT : (j + 1) * NT],
                                    start=(k == 0),
                                    stop=(k == KT - 1),
                                )
                        dst = lg[:, q * GW : (q + 1) * GW]
                        copy_idx += 1
                        if copy_idx % OFFLOAD_EVERY == 0:
                            nc.scalar.copy(dst, ps[:])
                        else:
                            nc.vector.tensor_copy(dst, ps[:])
                    nc.sync.dma_start(
                        logits[m * P : (m + 1) * P, s * SW : (s + 1) * SW], lg[:]
                    )
                    ex = epool.tile([P, SW], BF16, tag="ex")
                    nc.scalar.activation(
                        ex[:],
                        lg[:],
                        mybir.ActivationFunctionType.Exp,
                        bias=neg_scale[:],
                        scale=1.0,
                        accum_out=accums[m][:, s : s + 1],
                    )

                nc.vector.reduce_sum(
                    sums_sb[:, m : m + 1], accums[m][:], axis=mybir.AxisListType.X
                )
                nc.sync.dma_start(sums[:, m : m + 1], sums_sb[:, m : m + 1])

    nc.compile()
    return nc


def _get_nc(bl, c):
    key = (bl, c)
    if key not in _NC_CACHE:
        _NC_CACHE[key] = _build(bl, c)
    return _NC_CACHE[key]


def _run_device(x30, wn, bl, c):
    """x30: [nb, D] normalized*SCALE rows; wn: [c, D] normalized classes."""
    global LAST_RESULTS
    nc = _get_nc(bl, c)
    wT = np.ascontiguousarray(wn.T).astype(NP_BF16).reshape(KT, P, c)
    in_maps = []
    for i in range(NCORES):
        xi = (
            np.ascontiguousarray(x30[i * bl : (i + 1) * bl].T)
            .astype(NP_BF16)
            .reshape(KT, P, bl)
        )
        in_maps.append({"xT": xi, "wT": wT})
    res = run_bass_kernel_spmd(nc, in_maps, core_ids=list(range(NCORES)))
    LAST_RESULTS = res
    nb = bl * NCORES
    logits = np.empty((nb, c), np.float32)
    s_raw = np.empty(nb, np.float32)
    for i in range(NCORES):
        logits[i * bl : (i + 1) * bl] = res.results[i]["logits"].astype(
            np.float32
        )
        # sums[p, m] is row m*P+p of this core's shard
        s_raw[i * bl : (i + 1) * bl] = res.results[i]["sums"].T.reshape(-1)
    return logits, s_raw


def kernel(input, label, weight):
    x = np.asarray(input, dtype=np.float32)
    w = np.asarray(weight, dtype=np.float32)
    lab = np.asarray(label).astype(np.int64)

    xn = x / np.maximum(np.sqrt(np.sum(x * x, axis=1, keepdims=True)), EPS)
    wn = w / np.maximum(np.sqrt(np.sum(w * w, axis=1, keepdims=True)), EPS)
    x30 = (xn * np.float32(SCALE)).astype(np.float32)

    logits, s_raw = _run_device(x30, wn, x.shape[0] // NCORES, w.shape[0])

    nb = x.shape[0]
    rows = np.arange(nb)
    raw = logits[rows, lab].astype(np.float32)
    cosv = np.clip(raw / np.float32(SCALE), -1.0, 1.0)
    sinv = np.sqrt(np.maximum(np.float32(1.0) - cosv * cosv, 0.0))
    corr = (
        cosv * np.float32(np.cos(MARGIN)) - sinv * np.float32(np.sin(MARGIN))
    ) * np.float32(SCALE)
    logits[rows, lab] = corr
    s = s_raw + np.exp(corr - np.float32(SCALE)) - np.exp(raw - np.float32(SCALE))
    nll = np.float32(SCALE) + np.log(s) - corr
    loss = np.float32(np.mean(nll))
    return (np.asarray(loss, dtype=np.float32), logits)


# revision 23
# speedup vs baseline: 1.1983x; 1.1983x over previous
"""ArcFace loss on 8 Trainium2 NeuronCores.

Sharding: batch data-parallel. Each core gets 1024 rows of the (normalized,
pre-scaled, transposed) input, the full normalized transposed weight, and
computes its [1024, 16384] slab of logits plus per-row sum(exp(logit-30)).
Host assembles the slabs, applies the ArcFace margin at the 8192 label
positions, and finishes the (tiny) log-softmax NLL reduction.
"""

import os
import sys

import numpy as np

for _p in ("/opt/trn_rl_repo",):
    if os.path.isdir(_p) and _p not in sys.path:
        sys.path.insert(0, _p)

def _ensure_ntff_hook():
    """Register antenv.axon_hooks (absent in the trimmed repo) so that
    run_bass_kernel_spmd(trace=True) can NTFF-profile under axon."""
    try:
        from antenv.axon_hooks import get_axon_ntff_profile_hook  # noqa: F401

        return
    except ImportError:
        pass
    import types

    import antenv

    mod = types.ModuleType("antenv.axon_hooks")
    mod._hook = None

    def set_axon_ntff_profile_hook(h):
        mod._hook = h

    def get_axon_ntff_profile_hook():
        if mod._hook is None:
            try:
                from trn_agent_boot.trn_boot import _ntff_profile_via_ctypes

                mod._hook = _ntff_profile_via_ctypes("/opt/axon/libaxon_pjrt.so")
            except Exception:
                return None
        return mod._hook

    mod.set_axon_ntff_profile_hook = set_axon_ntff_profile_hook
    mod.get_axon_ntff_profile_hook = get_axon_ntff_profile_hook
    sys.modules["antenv.axon_hooks"] = mod
    antenv.axon_hooks = mod


_ensure_ntff_hook()

import concourse.bass as bass  # noqa: E402
import concourse.mybir as mybir  # noqa: E402
import concourse.tile as tile  # noqa: E402
from concourse import bacc  # noqa: E402
from concourse.bass_utils import run_bass_kernel_spmd  # noqa: E402

import ml_dtypes  # noqa: E402

B, C, D = 8192, 16384, 256
NCORES = 8
BL = B // NCORES  # 1024 rows per core
P = 128
KT = D // P  # 2 contraction tiles of 128
NT = 512  # matmul moving-operand max (PSUM f32: one bank per matmul)
GROUP = 4  # class tiles per PSUM group
GW = NT * GROUP  # 2048 = one PSUM tile / one DMA-out chunk
SCALE = 30.0
MARGIN = 0.3
EPS = 1e-12

F32 = mybir.dt.float32
BF16 = mybir.dt.bfloat16
NP_BF16 = ml_dtypes.bfloat16

LAST_RESULTS = None  # BassKernelResults of the most recent run (for profiling)

_NC_CACHE = {}


def _build(bl, c):
    """Build + compile the per-core Bass graph for a [bl, c] logits slab."""
    mt = bl // P
    ng = c // GW
    nc = bacc.Bacc(
        "TRN2", target_bir_lowering=False, debug=False, num_devices=NCORES
    )
    xT = nc.dram_tensor("xT", [KT, P, bl], BF16, kind="ExternalInput")
    wT = nc.dram_tensor("wT", [KT, P, c], BF16, kind="ExternalInput")
    logits = nc.dram_tensor("logits", [bl, c], BF16, kind="ExternalOutput")
    sums = nc.dram_tensor("sums", [P, mt], F32, kind="ExternalOutput")

    with tile.TileContext(nc) as tc:
        with (
            tc.tile_pool(name="w", bufs=1) as wpool,
            tc.tile_pool(name="x", bufs=1) as xpool,
            tc.tile_pool(name="lg", bufs=10) as lpool,
            tc.tile_pool(name="ex", bufs=3) as epool,
            tc.tile_pool(name="acc", bufs=1) as apool,
            tc.tile_pool(name="ps", bufs=2, space="PSUM") as pspool,
        ):
            xk = [xpool.tile([P, bl], BF16, tag=f"x{k}", name=f"x{k}") for k in range(KT)]
            for k in range(KT):
                nc.sync.dma_start(xk[k][:], xT[k])
            # Resident normalized-transposed weights: KT x ng chunks of [P, GW]
            wk = [
                [wpool.tile([P, GW], BF16, tag=f"w{k}_{g}", name=f"w{k}_{g}") for g in range(ng)]
                for k in range(KT)
            ]
            for g in range(ng):
                for k in range(KT):
                    if g == 0:
                        # finer chunks so the first matmuls start sooner
                        for j in range(GROUP):
                            nc.sync.dma_start(
                                wk[k][g][:, j * NT : (j + 1) * NT],
                                wT[k, :, g * GW + j * NT : g * GW + (j + 1) * NT],
                            )
                    else:
                        nc.sync.dma_start(
                            wk[k][g][:], wT[k, :, g * GW : (g + 1) * GW]
                        )

            neg_scale = apool.tile([P, 1], F32, tag="neg_scale")
            nc.vector.memset(neg_scale[:], -SCALE)

            sums_sb = apool.tile([P, mt], F32, tag="sums")
            # SLAB groups of GW columns form one exp/DMA slab of SW columns
            SLAB = 1
            SW = GW * SLAB  # 2048
            ns = c // SW  # slabs per row-block
            accums = [
                apool.tile([P, ns], F32, tag=f"accum{m}", name=f"accum{m}")
                for m in range(mt)
            ]

            # Offload a few PSUM->SBUF copies to ACT to balance DVE/ACT load
            copy_idx = 0
            OFFLOAD_EVERY = 43

            for s in range(ns):
                for m in range(mt):
                    lg = lpool.tile([P, SW], BF16, tag="lg")
                    for q in range(SLAB):
                        g = s * SLAB + q
                        ps = pspool.tile([P, GW], F32, tag="ps")
                        for j in range(GROUP):
                            for k in range(KT):
                                nc.tensor.matmul(
                                    ps[:, j * NT : (j + 1) * NT],
                                    lhsT=xk[k][:, m * P : (m + 1) * P],
                                    rhs=wk[k][g][:, j * NT : (j + 1) * NT],
                                    start=(k == 0),
                                    stop=(k == KT - 1),
                                )
                        dst = lg[:, q * GW : (q + 1) * GW]
                        copy_idx += 1
                        if copy_idx % OFFLOAD_EVERY == 0:
                            nc.scalar.copy(dst, ps[:])
                        else:
                            nc.vector.tensor_copy(dst, ps[:])
                    nc.sync.dma_start(
                        logits[m * P : (m + 1) * P, s * SW : (s + 1) * SW], lg[:]
                    )
                    ex = epool.tile([P, SW], BF16, tag="ex")
                    nc.scalar.activation(
                        ex[:],
                        lg[:],
                        mybir.ActivationFunctionType.Exp,
                        bias=neg_scale[:],
                        scale=1.0,
                        accum_out=accums[m][:, s : s + 1],
                    )

            for m in range(mt):
                nc.vector.reduce_sum(
                    sums_sb[:, m : m + 1], accums[m][:], axis=mybir.AxisListType.X
                )
            nc.sync.dma_start(sums[:], sums_sb[:])

    nc.compile()
    return nc


def _get_nc(bl, c):
    key = (bl, c)
    if key not in _NC_CACHE:
        _NC_CACHE[key] = _build(bl, c)
    return _NC_CACHE[key]


def _run_device(x30, wn, bl, c):
    """x30: [nb, D] normalized*SCALE rows; wn: [c, D] normalized classes."""
    global LAST_RESULTS
    nc = _get_nc(bl, c)
    wT = np.ascontiguousarray(wn.T).astype(NP_BF16).reshape(KT, P, c)
    in_maps = []
    for i in range(NCORES):
        xi = (
            np.ascontiguousarray(x30[i * bl : (i + 1) * bl].T)
            .astype(NP_BF16)
            .reshape(KT, P, bl)
        )
        in_maps.append({"xT": xi, "wT": wT})
    res = run_bass_kernel_spmd(nc, in_maps, core_ids=list(range(NCORES)))
    LAST_RESULTS = res
    nb = bl * NCORES
    logits = np.empty((nb, c), np.float32)
    s_raw = np.empty(nb, np.float32)
    for i in range(NCORES):
        logits[i * bl : (i + 1) * bl] = res.results[i]["logits"].astype(
            np.float32
        )
        # sums[p, m] is row m*P+p of this core's shard
        s_raw[i * bl : (i + 1) * bl] = res.results[i]["sums"].T.reshape(-1)
    return logits, s_raw


def kernel(input, label, weight):
    x = np.asarray(input, dtype=np.float32)
    w = np.asarray(weight, dtype=np.float32)
    lab = np.asarray(label).astype(np.int64)

    xn = x / np.maximum(np.sqrt(np.sum(x * x, axis=1, keepdims=True)), EPS)
    wn = w / np.maximum(np.sqrt(np.sum(w * w, axis=1, keepdims=True)), EPS)
    x30 = (xn * np.float32(SCALE)).astype(np.float32)

    logits, s_raw = _run_device(x30, wn, x.shape[0] // NCORES, w.shape[0])

    nb = x.shape[0]
    rows = np.arange(nb)
    raw = logits[rows, lab].astype(np.float32)
    cosv = np.clip(raw / np.float32(SCALE), -1.0, 1.0)
    sinv = np.sqrt(np.maximum(np.float32(1.0) - cosv * cosv, 0.0))
    corr = (
        cosv * np.float32(np.cos(MARGIN)) - sinv * np.float32(np.sin(MARGIN))
    ) * np.float32(SCALE)
    logits[rows, lab] = corr
    s = s_raw + np.exp(corr - np.float32(SCALE)) - np.exp(raw - np.float32(SCALE))
    nll = np.float32(SCALE) + np.log(s) - corr
    loss = np.float32(np.mean(nll))
    return (np.asarray(loss, dtype=np.float32), logits)
